# revision 24
# baseline (speedup 1.0000x reference)
"""TRN2 Bass kernel for nn_Attention_21758304322201 (sparse_attention).

Reference computation (B=32, L=2048, D=32, C=20):
    v = vals @ W_v.T
    k = LN(keys @ W_k.T);  q = LN(ques @ W_q.T)
    a = q @ k.T / sqrt(C);  a[masked keys] = -inf
    p = softmax(a);  o = p @ v
    out = LN(o + ques)

Strategy:
  * Data-parallel over batch: 4 batches per NeuronCore (8 cores).
  * Host-side (layout only): compact keys/vals to the unmasked set (padded
    to KC, a multiple of 128), transpose to [d, seq] layouts, pack the 4
    batches of a core into 32-row partition strips, build small constant
    matrices (augmented projection weights, strip indicators).
  * Device: everything is batched 4-ways through the PE array via
    tile_position row/col packing.  LN of q/k is folded algebraically into
    a 21-dim contraction (dim 20 carries the mean cross term) plus
    per-row/per-column rstd scalings.  Softmax has no max-subtraction
    (scores are bounded by ~sqrt(C)); the normalizer is obtained by M=1
    ones-stationary matmuls; division is folded into the output LN's scale
    invariance: LN(o/s + q) == LN(o + s*q).
  * exp() is split across ScalarE (native, exact) and VectorE (one-op
    Schraudolph: bf16 bit-pattern via int16(x*A+B)).
  * The only ACT table set used is natural_log_exp_and_others: rsqrt for
    both LNs is computed as exp(-0.5*ln(var+eps)).
"""
import math

import numpy as np

from concourse import bacc, bass, bass_utils, tile
from concourse import mybir

dt = mybir.dt
F32 = dt.float32
BF16 = dt.bfloat16
I16 = dt.int16
AO = mybir.AluOpType
AF = mybir.ActivationFunctionType

# problem constants (hardcoded per harness contract)
B, LQ, LK, D, C = 32, 2048, 2048, 32, 20
EPS = 1e-5
NCORES = 8
BPC = B // NCORES          # batches per core = 4
CAUG = C + 1               # 21-dim augmented contraction
NT = 256                   # q-tile width
NQT = LQ // NT             # 4 q tiles

# int16 Schraudolph (bf16 bit pattern): bits = round(x * A16 + B16)
A16 = 128.0 / math.log(2.0)
B16 = 127.0 * 128.0 - 5.6          # tuned: max rel err 3.3%, mean 1.8%
B16_PAD = 16.0 * 128.0             # bf16 2^-111: pad keys contribute ~0
ACT_PAD_BIAS = -60.0               # exp(-60) == 0 for padded keys on ACT

# exp engine split: j-chunks assigned to DVE (rest go to ACT)
DVE_CHUNKS = frozenset({1, 4, 7})
DEBUG = False

_cache: dict = {}


def _assign_dve(qt: int, j: int, b: int) -> bool:
    return j in DVE_CHUNKS


def build_module(KC: int, reps: int = 1):
    """Build the SPMD bass module for per-core work. KC = padded key count."""
    NJ = KC // 128
    nc = bacc.Bacc("TRN2", target_bir_lowering=False, debug=False,
                   num_devices=NCORES)

    def din(name, shape):
        return nc.dram_tensor(name, shape, F32, kind="ExternalInput").ap()

    quesT_d = din("quesT", [128, LQ])
    keysT_d = din("keysT", [128, KC])
    valsP_d = din("valsP", [128, NJ * 128])
    wq_d = din("wq_st", [128, CAUG])
    wk_d = din("wk_st", [128, CAUG])
    wv_d = din("wv_st", [128, D])
    indsig_d = din("ind_sig", [128, BPC])
    indsq_d = din("ind_sq", [128, BPC])
    indb_d = din("ind_b", [128, BPC])
    guard_d = din("guard", [128, BPC * NJ])
    gobo_d = din("go_bo", [128, 2])
    ones_d = din("ones_in", [128, 32])
    out_d = nc.dram_tensor("out", [128, LQ], F32, kind="ExternalOutput").ap()
    dbg = {}
    if False and DEBUG:
        for nm, shape in [("qsc", [128, LQ]), ("khat", [128, KC]),
                          ("rkcols", [128, 4 * (KC // 128)]),
                          ("scact", [128, 4 * (KC // 128)]),
                          ("p00", [128, NT]), ("oacc", [128, NT]),
                          ("sacc", [128, NT]), ("z", [128, NT]),
                          ("var", [4, NT]), ("rstdo", [4, NT]),
                          ("kvar", [4, KC]), ("ksig", [4, KC]),
                          ("ksq", [4, KC]), ("krstd", [4, KC])]:
            dbg[nm] = nc.dram_tensor("dbg_" + nm, shape, F32,
                                     kind="ExternalOutput").ap()

    GS = 1.0 / math.sqrt(C)  # global score scale (uniform g folded in host-side)
    KCv = KC

    with tile.TileContext(nc) as tc:
        with tc.tile_pool(name="inp", bufs=1) as inp, \
             tc.tile_pool(name="cst", bufs=1) as cst, \
             tc.tile_pool(name="big", bufs=1) as big, \
             tc.tile_pool(name="sml", bufs=1) as sml:
            # ---- load inputs ----
            quesT = inp.tile([128, LQ], F32)
            nc.sync.dma_start(quesT[:], quesT_d)
            keysT = inp.tile([128, KC], F32)
            nc.sync.dma_start(keysT[:], keysT_d)
            valsP = inp.tile([128, NJ, 128], F32)
            nc.sync.dma_start(valsP[:], valsP_d.rearrange("p (j c) -> p j c", j=NJ))
            wq_f = cst.tile([128, CAUG], F32)
            nc.sync.dma_start(wq_f[:], wq_d)
            wk_f = cst.tile([128, CAUG], F32)
            nc.sync.dma_start(wk_f[:], wk_d)
            wv_f = cst.tile([128, D], F32)
            nc.sync.dma_start(wv_f[:], wv_d)
            indsig_f = cst.tile([128, BPC], F32)
            nc.sync.dma_start(indsig_f[:], indsig_d)
            indsq_f = cst.tile([128, BPC], F32)
            nc.sync.dma_start(indsq_f[:], indsq_d)
            indb_f = cst.tile([128, BPC], F32)
            nc.sync.dma_start(indb_f[:], indb_d)
            guard = cst.tile([128, BPC * NJ], F32)
            nc.sync.dma_start(guard[:], guard_d)
            gobo = cst.tile([128, 2], F32)
            nc.sync.dma_start(gobo[:], gobo_d)

            # ---- constant conversions to bf16 ----
            wq_bf = cst.tile([128, CAUG], BF16)
            nc.vector.tensor_copy(wq_bf[:], wq_f[:])
            wk_bf = cst.tile([128, CAUG], BF16)
            nc.vector.tensor_copy(wk_bf[:], wk_f[:])
            wv_bf = cst.tile([128, D], BF16)
            nc.vector.tensor_copy(wv_bf[:], wv_f[:])
            indsig_bf = cst.tile([128, BPC], BF16)
            nc.vector.tensor_copy(indsig_bf[:], indsig_f[:])
            indsq_bf = cst.tile([128, BPC], BF16)
            nc.vector.tensor_copy(indsq_bf[:], indsq_f[:])
            indb_bf = cst.tile([128, BPC], BF16)
            nc.vector.tensor_copy(indb_bf[:], indb_f[:])
            ones_f = cst.tile([128, 32], F32)
            nc.sync.dma_start(ones_f[:], ones_d)
            ones_bf = cst.tile([128, 32], BF16)
            nc.vector.tensor_copy(ones_bf[:], ones_f[:])
            eps_t = cst.tile([4, 1], F32)
            nc.gpsimd.memset(eps_t[:], EPS)

            def body(_iv=None):
                _body(nc, tc, locals_pack)

            # pack everything the body needs
            locals_pack = dict(
                NJ=NJ, quesT=quesT, keysT=keysT, valsP=valsP,
                wq_bf=wq_bf, wk_bf=wk_bf, wv_bf=wv_bf,
                indsig_bf=indsig_bf, indsq_bf=indsq_bf, indb_bf=indb_bf,
                ones_bf=ones_bf, guard=guard, gobo=gobo, out_d=out_d, GS=GS,
                dbg=dbg,
                eps_t=eps_t,
            )

            with tc.tile_pool(name="epdram", bufs=1, space="DRAM") as epdram_pool:
                ep_dram_t = epdram_pool.tile([3, 4, LQ], F32, tag="epdram")
                locals_pack["ep_dram"] = ep_dram_t
                if reps == 1:
                    body()
                else:
                    with tc.For_i(0, reps, 1):
                        body()

    nc.compile()
    return nc


def _body(nc, tc, pk):
    """One full forward pass for this core's 4 batches."""
    NJ = pk["NJ"]
    KC = NJ * 128
    quesT, keysT, valsP = pk["quesT"], pk["keysT"], pk["valsP"]
    wq_bf, wk_bf, wv_bf = pk["wq_bf"], pk["wk_bf"], pk["wv_bf"]
    indsig_bf, indsq_bf, indb_bf = pk["indsig_bf"], pk["indsq_bf"], pk["indb_bf"]
    ones_bf, guard, gobo, out_d, GS = (
        pk["ones_bf"], pk["guard"], pk["gobo"], pk["out_d"], pk["GS"])
    eps_t = pk["eps_t"]
    ep_dram = pk["ep_dram"]

    with tc.tile_pool(name="work", bufs=1) as wk:

        # ================= phase 1: projections + LN stats =================
        with tc.tile_pool(name="ph1sb", bufs=1) as sb1:
            quesT_bf = wk.tile([128, LQ], BF16)
            nc.vector.tensor_copy(quesT_bf[:], quesT[:])
            keysT_bf = sb1.tile([128, KC], BF16)
            nc.vector.tensor_copy(keysT_bf[:], keysT[:])
            valsP_bf = wk.tile([128, NJ, 128], BF16)
            nc.vector.tensor_copy(valsP_bf[:], valsP[:])

            def proj_stats(src_bf, W_bf, L, sig_scale, tg):
                """Row-packed projection; returns (proj_bf, var rows [4, L])."""
                with tc.tile_pool(name=f"pps{tg}", bufs=1, space="PSUM") as ps1:
                    proj_ps = ps1.tile([128, L], F32, tag=f"proj{tg}")
                    nc.vector.memset(proj_ps[:], 0.0)
                    for b in range(4):
                        for t0 in range(0, L, 512):
                            w = min(512, L - t0)
                            nc.tensor.matmul(
                                proj_ps[32 * b:32 * b + CAUG, t0:t0 + w],
                                W_bf[32 * b:32 * b + D, :],
                                src_bf[32 * b:32 * b + D, t0:t0 + w],
                                start=True, stop=True,
                                tile_position=(32 * b, 32 * b),
                            )
                    proj_bf = wk.tile([128, L], BF16, tag=f"projbf{tg}")
                    nc.vector.tensor_copy(proj_bf[:], proj_ps[:])
                sq_bf = sb1.tile([128, L], BF16, tag=f"sq{tg}")
                nc.vector.tensor_tensor(sq_bf[:], proj_bf[:], proj_bf[:], AO.mult)
                with tc.tile_pool(name=f"sps{tg}", bufs=1, space="PSUM") as ps2:
                    stat_ps = ps2.tile([64, L], F32, tag=f"stat{tg}")
                    for t0 in range(0, L, 512):
                        w = min(512, L - t0)
                        nc.tensor.matmul(stat_ps[0:4, t0:t0 + w],
                                         indsig_bf[:], proj_bf[:, t0:t0 + w],
                                         start=True, stop=True)
                        nc.tensor.matmul(stat_ps[32:36, t0:t0 + w],
                                         indsq_bf[:], sq_bf[:, t0:t0 + w],
                                         start=True, stop=True,
                                         tile_position=(0, 32))
                    rows_sig = sb1.tile([4, L], F32, tag=f"rsig{tg}")
                    nc.scalar.copy(rows_sig[:], stat_ps[0:4, :])
                    rows_sq = sb1.tile([4, L], F32, tag=f"rsq{tg}")
                    nc.scalar.copy(rows_sq[:], stat_ps[32:36, :])
                mu = sb1.tile([4, L], F32, tag=f"mu{tg}")
                nc.scalar.mul(mu[:], rows_sig[:], sig_scale / C)
                musq = sb1.tile([4, L], F32, tag=f"musq{tg}")
                nc.vector.tensor_tensor(musq[:], mu[:], mu[:], AO.mult)
                var = sb1.tile([4, L], F32, tag=f"var{tg}")
                nc.vector.scalar_tensor_tensor(
                    var[:], rows_sq[:], 1.0 / C, musq[:], AO.mult, AO.subtract)
                return proj_bf, var

            qhat_bf, var_q = proj_stats(quesT_bf, wq_bf, LQ, 1.0, "q")
            khat_bf, var_k = proj_stats(keysT_bf, wk_bf, KC, -1.0, "k")
            pk["khat_bf"] = khat_bf

            # batched Ln then batched Exp (2 ACT table loads total)
            lnq = sb1.tile([4, LQ], F32)
            nc.scalar.activation(lnq[:], var_q[:], AF.Ln, bias=eps_t[:])
            lnk = sb1.tile([4, KC], F32)
            nc.scalar.activation(lnk[:], var_k[:], AF.Ln, bias=eps_t[:])
            rq_bf = sb1.tile([4, LQ], BF16)
            nc.scalar.activation(rq_bf[:], lnq[:], AF.Exp, scale=-0.5)
            rk_f = sb1.tile([4, KC], F32)
            nc.scalar.activation(rk_f[:], lnk[:], AF.Exp, scale=-0.5)

            # broadcast r_q to strips via DRAM bounce; fold into q side
            rq_bc = sb1.tile([128, LQ], BF16)
            with tc.tile_pool(name="dramq", bufs=1, space="DRAM") as dramq:
                rq_dram = dramq.tile([4, LQ], BF16, tag="rqd")
                nc.sync.dma_start(rq_dram[:], rq_bf[0:4, :])
                for b in range(4):
                    nc.sync.dma_start(
                        rq_bc[32 * b:32 * b + CAUG, :],
                        rq_dram[b:b + 1, :].broadcast_to([CAUG, LQ]))
            qsc_bf = wk.tile([128, LQ], BF16)
            nc.vector.tensor_tensor(qsc_bf[:], qhat_bf[:], rq_bc[:], AO.mult)

            # r_k rows -> per-chunk column layout via DRAM bounce
            rk_cols = wk.tile([128, 4 * NJ], F32, tag="rk_cols")
            with tc.tile_pool(name="dramsc", bufs=1, space="DRAM") as dramp:
                rk_dram = dramp.tile([4, KC], F32, tag="rkd")
                nc.sync.dma_start(rk_dram[:], rk_f[0:4, :])
                for b in range(4):
                    nc.sync.dma_start(
                        rk_cols[:, NJ * b:NJ * (b + 1)],
                        rk_dram[b].rearrange("(j p) -> p j", p=128))
            sc_act = wk.tile([128, 4 * NJ], F32)   # rk * GS * guard
            nc.vector.tensor_tensor(sc_act[:], rk_cols[:], guard[:], AO.mult)
            nc.vector.tensor_scalar(sc_act[:], sc_act[:], GS, None, AO.mult)
            bias_act = wk.tile([128, 4 * NJ], F32)  # 0 real / -60 pad
            nc.vector.tensor_scalar(bias_act[:], guard[:], -ACT_PAD_BIAS,
                                    ACT_PAD_BIAS, AO.mult, AO.add)
            sc_dve = wk.tile([128, 4 * NJ], F32)   # rk * GS * A16 * guard
            nc.vector.tensor_scalar(sc_dve[:], sc_act[:], A16, None, AO.mult)
            b_dve = wk.tile([128, 4 * NJ], F32)    # B16 real / B16_PAD pad
            nc.vector.tensor_scalar(b_dve[:], guard[:], B16 - B16_PAD, B16_PAD,
                                    AO.mult, AO.add)

        # ================= phase 2: attention =================
        o_bfs = []
        obfp = wk  # o_bf tiles are consumed in phase 3; keep them body-scoped
        with tc.tile_pool(name="scps", bufs=4, space="PSUM") as scps, \
             tc.tile_pool(name="oacc", bufs=2, space="PSUM") as oaccp, \
             tc.tile_pool(name="sacc", bufs=2, space="PSUM") as saccp, \
             tc.tile_pool(name="psb", bufs=10) as psb, \
             tc.tile_pool(name="sumsp", bufs=2) as sumsp:
            for qt in range(NQT):
                t0 = qt * NT
                o_acc = oaccp.tile([128, NT], F32, tag="o")
                s_acc = saccp.tile([128, NT], F32, tag="s")
                for j in range(NJ):
                    # two scores banks, each holds 2 batches at free offsets
                    sc0 = scps.tile([128, 2 * NT], F32, tag="sc")
                    sc1 = scps.tile([128, 2 * NT], F32, tag="sc")
                    sc_slices = [sc0[:, 0:NT], sc0[:, NT:2 * NT],
                                 sc1[:, 0:NT], sc1[:, NT:2 * NT]]
                    p_tiles = []
                    for b in range(4):
                        s_ps = sc_slices[b]
                        nc.tensor.matmul(
                            s_ps,
                            pk["khat_bf"][32 * b:32 * b + CAUG,
                                          128 * j:128 * (j + 1)],
                            qsc_bf[32 * b:32 * b + CAUG, t0:t0 + NT],
                            start=True, stop=True,
                            tile_position=(32 * b, 0),
                        )
                        col = NJ * b + j
                        if _assign_dve(qt, j, b):
                            p_i16 = psb.tile([128, NT], I16, tag="p")
                            nc.vector.tensor_scalar(
                                p_i16[:], s_ps,
                                sc_dve[:, col:col + 1], b_dve[:, col:col + 1],
                                AO.mult, AO.add)
                            p_bf = p_i16[:].bitcast(BF16)
                        else:
                            p_t = psb.tile([128, NT], BF16, tag="p")
                            nc.scalar.activation(
                                p_t[:], s_ps, AF.Exp,
                                bias=bias_act[:, col:col + 1],
                                scale=sc_act[:, col:col + 1])
                            p_bf = p_t[:]
                        p_tiles.append(p_bf)
                    st, sp = (j == 0), (j == NJ - 1)
                    for b in range(4):
                        nc.tensor.matmul(
                            o_acc[32 * b:32 * b + 32, :],
                            valsP_bf[:, j, 32 * b:32 * b + 32],
                            p_tiles[b],
                            start=st, stop=sp, tile_position=(0, 32 * b))
                    for b in range(4):
                        nc.tensor.matmul(
                            s_acc[32 * b:32 * b + 32, :],
                            ones_bf[:],
                            p_tiles[b],
                            start=st, stop=sp, tile_position=(0, 32 * b))

                # stash o (bf16) + sums rows (via DRAM) for the finalize phase
                o_bf = obfp.tile([128, NT], BF16, tag=f"obf{qt}")
                nc.vector.tensor_copy(o_bf[:], o_acc[:])
                o_bfs.append(o_bf)
                sums = sumsp.tile([128, NT], F32, tag="sums")
                nc.scalar.copy(sums[:], s_acc[:])
                for b in range(4):
                    nc.sync.dma_start(ep_dram[0, b:b + 1, t0:t0 + NT],
                                      sums[32 * b:32 * b + 1, :])

        # ================= phase 3: output LN finalize =================
        with tc.tile_pool(name="ep", bufs=2) as ep, \
             tc.tile_pool(name="zp", bufs=NQT + 1) as zp, \
             tc.tile_pool(name="eprow", bufs=1) as eprow, \
             tc.tile_pool(name="epps", bufs=2, space="PSUM") as epps, \
             tc.tile_pool(name="stps", bufs=2, space="PSUM") as stps:
            zs = []
            srow_z = eprow.tile([4, LQ], F32)
            srow_z2 = eprow.tile([4, LQ], F32)
            for qt in range(NQT):
                t0 = qt * NT
                z1_ps = epps.tile([128, NT], F32, tag="z1")
                for b in range(4):
                    nc.tensor.matmul(
                        z1_ps[32 * b:32 * b + 32, :],
                        wv_bf[32 * b:32 * b + 32, :],
                        o_bfs[qt][32 * b:32 * b + 32, :],
                        start=True, stop=True,
                        tile_position=(32 * b, 32 * b))
                z1 = ep.tile([128, NT], F32, tag="z1sb")
                nc.scalar.copy(z1[:], z1_ps[:])
                s_bc = ep.tile([128, NT], F32, tag="sbc")
                for b in range(4):
                    nc.sync.dma_start(
                        s_bc[32 * b:32 * b + 32, :],
                        ep_dram[0, b:b + 1, t0:t0 + NT].broadcast_to([32, NT]))
                t1 = ep.tile([128, NT], F32, tag="t1")
                nc.vector.tensor_tensor(t1[:], quesT[:, t0:t0 + NT], s_bc[:],
                                        AO.mult)
                z = zp.tile([128, NT], F32, tag="z")
                nc.vector.tensor_tensor(z[:], t1[:], z1[:], AO.add)
                zs.append(z)
                z_bf = ep.tile([128, NT], BF16, tag="zbf")
                nc.vector.tensor_copy(z_bf[:], z[:])
                zsq_bf = ep.tile([128, NT], BF16, tag="zsq")
                nc.vector.tensor_tensor(zsq_bf[:], z_bf[:], z_bf[:], AO.mult)
                st_ps = stps.tile([64, NT], F32, tag="st")
                nc.tensor.matmul(st_ps[0:4, :], indb_bf[:], z_bf[:],
                                 start=True, stop=True)
                nc.tensor.matmul(st_ps[32:36, :], indb_bf[:], zsq_bf[:],
                                 start=True, stop=True, tile_position=(0, 32))
                nc.scalar.copy(srow_z[:, t0:t0 + NT], st_ps[0:4, :])
                nc.scalar.copy(srow_z2[:, t0:t0 + NT], st_ps[32:36, :])

            mu = eprow.tile([4, LQ], F32)
            nc.scalar.mul(mu[:], srow_z[:], 1.0 / D)
            musq = eprow.tile([4, LQ], F32)
            nc.vector.tensor_tensor(musq[:], mu[:], mu[:], AO.mult)
            var = eprow.tile([4, LQ], F32)
            nc.vector.scalar_tensor_tensor(
                var[:], srow_z2[:], 1.0 / D, musq[:], AO.mult, AO.subtract)
            lnv = eprow.tile([4, LQ], F32)
            nc.scalar.activation(lnv[:], var[:], AF.Ln, bias=eps_t[:])
            rstd = eprow.tile([4, LQ], F32)
            nc.scalar.activation(rstd[:], lnv[:], AF.Exp, scale=-0.5)
            nc.sync.dma_start(ep_dram[1, :, :], mu[:])
            nc.sync.dma_start(ep_dram[2, :, :], rstd[:])
            for qt in range(NQT):
                t0 = qt * NT
                mu_bc = ep.tile([128, NT], F32, tag="mubc")
                rstd_bc = ep.tile([128, NT], F32, tag="rstdbc")
                for b in range(4):
                    nc.sync.dma_start(
                        mu_bc[32 * b:32 * b + 32, :],
                        ep_dram[1, b:b + 1, t0:t0 + NT].broadcast_to([32, NT]))
                    nc.sync.dma_start(
                        rstd_bc[32 * b:32 * b + 32, :],
                        ep_dram[2, b:b + 1, t0:t0 + NT].broadcast_to([32, NT]))
                d1 = ep.tile([128, NT], F32, tag="d1")
                nc.vector.tensor_tensor(d1[:], zs[qt][:], mu_bc[:], AO.subtract)
                d2 = ep.tile([128, NT], F32, tag="d2")
                nc.vector.tensor_tensor(d2[:], d1[:], rstd_bc[:], AO.mult)
                zo = ep.tile([128, NT], F32, tag="zo")
                nc.vector.tensor_scalar(zo[:], d2[:], gobo[:, 0:1], gobo[:, 1:2],
                                        AO.mult, AO.add)
                nc.sync.dma_start(out_d[:, t0:t0 + NT], zo[:])


# ---------------------------------------------------------------------------
# host side
# ---------------------------------------------------------------------------

def prepare_inputs(vals, keys, ques, key_mask, W_v, W_k, W_q,
                   g_k, b_k, g_q, b_q, g_o, b_o):
    """Shard + lay out the full inputs for the 8 cores. Returns (in_maps, KC)."""
    vals = np.ascontiguousarray(vals, np.float32)
    keys = np.ascontiguousarray(keys, np.float32)
    ques = np.ascontiguousarray(ques, np.float32)
    key_mask = np.asarray(key_mask)
    W_v = np.asarray(W_v, np.float32)
    W_k = np.asarray(W_k, np.float32)
    W_q = np.asarray(W_q, np.float32)
    g_k = np.asarray(g_k, np.float32)
    b_k = np.asarray(b_k, np.float32)
    g_q = np.asarray(g_q, np.float32)
    b_q = np.asarray(b_q, np.float32)
    g_o = np.asarray(g_o, np.float32)
    b_o = np.asarray(b_o, np.float32)

    # supported parameterization (holds for the harness inputs)
    if not (np.allclose(b_k, 0) and np.allclose(b_q, 0)):
        raise NotImplementedError("nonzero k/q LN bias not supported")
    if not (np.allclose(g_k, g_k.flat[0]) and np.allclose(g_q, g_q.flat[0])):
        raise NotImplementedError("non-uniform k/q LN gain not supported")
    guni = float(g_k.flat[0] * g_q.flat[0])

    counts = (~key_mask).sum(axis=1)
    KC = int(np.ceil(max(int(counts.max()), 1) / 128) * 128)
    NJ = KC // 128

    s20 = math.sqrt(C)
    wq_aug = np.zeros((32, CAUG), np.float32)
    wq_aug[:, :C] = W_q.T
    wq_aug[:, C] = W_q.sum(axis=0) / s20
    wk_aug = np.zeros((32, CAUG), np.float32)
    wk_aug[:, :C] = W_k.T
    wk_aug[:, C] = -W_k.sum(axis=0) / s20

    wq_st = np.zeros((128, CAUG), np.float32)
    wk_st = np.zeros((128, CAUG), np.float32)
    wv_st = np.zeros((128, D), np.float32)
    indsig = np.zeros((128, BPC), np.float32)
    indsq = np.zeros((128, BPC), np.float32)
    indb = np.zeros((128, BPC), np.float32)
    go_bo = np.zeros((128, 2), np.float32)
    for b in range(BPC):
        wq_st[32 * b:32 * b + 32] = wq_aug
        wk_st[32 * b:32 * b + 32] = wk_aug
        wv_st[32 * b:32 * b + 32] = W_v.T
        indsig[32 * b + C, b] = s20
        indsq[32 * b:32 * b + C, b] = 1.0
        indb[32 * b:32 * b + 32, b] = 1.0
        go_bo[32 * b:32 * b + 32, 0] = g_o
        go_bo[32 * b:32 * b + 32, 1] = b_o
    # fold uniform gain into the score scale via wq (GS stays 1/sqrt(C))
    wq_st *= guni

    in_maps = []
    for c in range(NCORES):
        quesT = np.zeros((128, LQ), np.float32)
        keysT = np.zeros((128, KC), np.float32)
        valsP = np.zeros((128, NJ * 128), np.float32)
        guard = np.zeros((128, BPC * NJ), np.float32)
        for b in range(BPC):
            g = c * BPC + b
            idx = np.flatnonzero(~key_mask[g])
            ci = len(idx)
            quesT[32 * b:32 * b + 32] = ques[g].T
            keysT[32 * b:32 * b + 32, :ci] = keys[g][idx].T
            vc = np.zeros((KC, D), np.float32)
            vc[:ci] = vals[g][idx]
            for j in range(NJ):
                valsP[:, 128 * j + 32 * b:128 * j + 32 * b + 32] = \
                    vc[128 * j:128 * (j + 1)]
            gcol = np.zeros((KC,), np.float32)
            gcol[:ci] = 1.0
            guard[:, NJ * b:NJ * (b + 1)] = gcol.reshape(NJ, 128).T
        in_maps.append({
            "quesT": quesT, "keysT": keysT, "valsP": valsP,
            "wq_st": wq_st, "wk_st": wk_st, "wv_st": wv_st,
            "ind_sig": indsig, "ind_sq": indsq, "ind_b": indb,
            "guard": guard, "go_bo": go_bo,
            "ones_in": np.concatenate([np.ones((128, 1), np.float32),
                                       np.zeros((128, 31), np.float32)], axis=1),
        })
    return in_maps, KC


def unshard_output(results):
    out = np.empty((B, LQ, D), np.float32)
    for c in range(NCORES):
        o = results[c]["out"]
        for b in range(BPC):
            out[c * BPC + b] = o[32 * b:32 * b + 32, :].T
    return out


def kernel(**inputs) -> np.ndarray:
    in_maps, KC = prepare_inputs(**inputs)
    key = ("nc", KC)
    if key not in _cache:
        _cache[key] = build_module(KC)
    nc = _cache[key]
    res = bass_utils.run_bass_kernel_spmd(nc, in_maps,
                                          core_ids=list(range(NCORES)))
    return unshard_output(res.results)


# revision 25
# speedup vs baseline: 1.0212x; 1.0212x over previous
"""TRN2 Bass kernel for nn_Attention_21758304322201 (sparse_attention).

Reference computation (B=32, L=2048, D=32, C=20):
    v = vals @ W_v.T
    k = LN(keys @ W_k.T);  q = LN(ques @ W_q.T)
    a = q @ k.T / sqrt(C);  a[masked keys] = -inf
    p = softmax(a);  o = p @ v
    out = LN(o + ques)

Strategy:
  * Data-parallel over batch: 4 batches per NeuronCore (8 cores).
  * Host-side (layout only): compact keys/vals to the unmasked set (padded
    to KC, a multiple of 128), transpose to [d, seq] layouts, pack the 4
    batches of a core into 32-row partition strips, build small constant
    matrices (augmented projection weights, strip indicators).
  * Device: everything is batched 4-ways through the PE array via
    tile_position row/col packing.  LN of q/k is folded algebraically into
    a 21-dim contraction (dim 20 carries the mean cross term) plus
    per-row/per-column rstd scalings.  Softmax has no max-subtraction
    (scores are bounded by ~sqrt(C)); the normalizer is obtained by M=1
    ones-stationary matmuls; division is folded into the output LN's scale
    invariance: LN(o/s + q) == LN(o + s*q).
  * exp() is split across ScalarE (native, exact) and VectorE (one-op
    Schraudolph: bf16 bit-pattern via int16(x*A+B)).
  * The only ACT table set used is natural_log_exp_and_others: rsqrt for
    both LNs is computed as exp(-0.5*ln(var+eps)).
"""
import math

import numpy as np

from concourse import bacc, bass, bass_utils, tile
from concourse import mybir

dt = mybir.dt
F32 = dt.float32
BF16 = dt.bfloat16
I16 = dt.int16
AO = mybir.AluOpType
AF = mybir.ActivationFunctionType

# problem constants (hardcoded per harness contract)
B, LQ, LK, D, C = 32, 2048, 2048, 32, 20
EPS = 1e-5
NCORES = 8
BPC = B // NCORES          # batches per core = 4
CAUG = C + 1               # 21-dim augmented contraction
NT = 256                   # q-tile width
NQT = LQ // NT             # 4 q tiles

# int16 Schraudolph (bf16 bit pattern): bits = round(x * A16 + B16)
A16 = 128.0 / math.log(2.0)
B16 = 127.0 * 128.0 - 5.6          # tuned: max rel err 3.3%, mean 1.8%
B16_PAD = 16.0 * 128.0             # bf16 2^-111: pad keys contribute ~0
ACT_PAD_BIAS = -60.0               # exp(-60) == 0 for padded keys on ACT

# exp engine split: j-chunks assigned to DVE (rest go to ACT)
DVE_CHUNKS = frozenset({1, 4, 7})
DEBUG = False

_cache: dict = {}


def _assign_dve(qt: int, j: int, b: int) -> bool:
    return j in DVE_CHUNKS


def build_module(KC: int, reps: int = 1):
    """Build the SPMD bass module for per-core work. KC = padded key count."""
    NJ = KC // 128
    nc = bacc.Bacc("TRN2", target_bir_lowering=False, debug=False,
                   num_devices=NCORES)

    def din(name, shape):
        return nc.dram_tensor(name, shape, F32, kind="ExternalInput").ap()

    quesT_d = din("quesT", [128, LQ])
    keysT_d = din("keysT", [128, KC])
    valsP_d = din("valsP", [128, NJ * 128])
    wq_d = din("wq_st", [128, CAUG])
    wk_d = din("wk_st", [128, CAUG])
    wv_d = din("wv_st", [128, D])
    indsig_d = din("ind_sig", [128, BPC])
    indsq_d = din("ind_sq", [128, BPC])
    indb_d = din("ind_b", [128, BPC])
    guard_d = din("guard", [128, BPC * NJ])
    gobo_d = din("go_bo", [128, 2])
    ones_d = din("ones_in", [128, 32])
    out_d = nc.dram_tensor("out", [128, LQ], F32, kind="ExternalOutput").ap()
    dbg = {}
    if False and DEBUG:
        for nm, shape in [("qsc", [128, LQ]), ("khat", [128, KC]),
                          ("rkcols", [128, 4 * (KC // 128)]),
                          ("scact", [128, 4 * (KC // 128)]),
                          ("p00", [128, NT]), ("oacc", [128, NT]),
                          ("sacc", [128, NT]), ("z", [128, NT]),
                          ("var", [4, NT]), ("rstdo", [4, NT]),
                          ("kvar", [4, KC]), ("ksig", [4, KC]),
                          ("ksq", [4, KC]), ("krstd", [4, KC])]:
            dbg[nm] = nc.dram_tensor("dbg_" + nm, shape, F32,
                                     kind="ExternalOutput").ap()

    GS = 1.0 / math.sqrt(C)  # global score scale (uniform g folded in host-side)
    KCv = KC

    with tile.TileContext(nc) as tc:
        with tc.tile_pool(name="inp", bufs=1) as inp, \
             tc.tile_pool(name="cst", bufs=1) as cst, \
             tc.tile_pool(name="big", bufs=1) as big, \
             tc.tile_pool(name="sml", bufs=1) as sml:
            # ---- load inputs ----
            quesT = inp.tile([128, LQ], F32)
            nc.sync.dma_start(quesT[:], quesT_d)
            keysT = inp.tile([128, KC], F32)
            nc.sync.dma_start(keysT[:], keysT_d)
            valsP = inp.tile([128, NJ, 128], F32)
            nc.sync.dma_start(valsP[:], valsP_d.rearrange("p (j c) -> p j c", j=NJ))
            wq_f = cst.tile([128, CAUG], F32)
            nc.sync.dma_start(wq_f[:], wq_d)
            wk_f = cst.tile([128, CAUG], F32)
            nc.sync.dma_start(wk_f[:], wk_d)
            wv_f = cst.tile([128, D], F32)
            nc.sync.dma_start(wv_f[:], wv_d)
            indsig_f = cst.tile([128, BPC], F32)
            nc.sync.dma_start(indsig_f[:], indsig_d)
            indsq_f = cst.tile([128, BPC], F32)
            nc.sync.dma_start(indsq_f[:], indsq_d)
            indb_f = cst.tile([128, BPC], F32)
            nc.sync.dma_start(indb_f[:], indb_d)
            guard = cst.tile([128, BPC * NJ], F32)
            nc.sync.dma_start(guard[:], guard_d)
            gobo = cst.tile([128, 2], F32)
            nc.sync.dma_start(gobo[:], gobo_d)

            # ---- constant conversions to bf16 ----
            wq_bf = cst.tile([128, CAUG], BF16)
            nc.vector.tensor_copy(wq_bf[:], wq_f[:])
            wk_bf = cst.tile([128, CAUG], BF16)
            nc.vector.tensor_copy(wk_bf[:], wk_f[:])
            wv_bf = cst.tile([128, D], BF16)
            nc.vector.tensor_copy(wv_bf[:], wv_f[:])
            indsig_bf = cst.tile([128, BPC], BF16)
            nc.vector.tensor_copy(indsig_bf[:], indsig_f[:])
            indsq_bf = cst.tile([128, BPC], BF16)
            nc.vector.tensor_copy(indsq_bf[:], indsq_f[:])
            indb_bf = cst.tile([128, BPC], BF16)
            nc.vector.tensor_copy(indb_bf[:], indb_f[:])
            ones_f = cst.tile([128, 32], F32)
            nc.sync.dma_start(ones_f[:], ones_d)
            ones_bf = cst.tile([128, 32], BF16)
            nc.vector.tensor_copy(ones_bf[:], ones_f[:])
            eps_t = cst.tile([4, 1], F32)
            nc.gpsimd.memset(eps_t[:], EPS)

            def body(_iv=None):
                _body(nc, tc, locals_pack)

            # pack everything the body needs
            locals_pack = dict(
                NJ=NJ, quesT=quesT, keysT=keysT, valsP=valsP,
                wq_bf=wq_bf, wk_bf=wk_bf, wv_bf=wv_bf,
                indsig_bf=indsig_bf, indsq_bf=indsq_bf, indb_bf=indb_bf,
                ones_bf=ones_bf, guard=guard, gobo=gobo, out_d=out_d, GS=GS,
                dbg=dbg,
                eps_t=eps_t,
            )

            with tc.tile_pool(name="epdram", bufs=1, space="DRAM") as epdram_pool:
                ep_dram_t = epdram_pool.tile([3, 4, LQ], F32, tag="epdram")
                locals_pack["ep_dram"] = ep_dram_t
                if reps == 1:
                    body()
                elif reps == 0:
                    pass
                else:
                    with tc.For_i(0, reps, 1):
                        body()

    nc.compile()
    return nc


def _body(nc, tc, pk):
    """One full forward pass for this core's 4 batches."""
    NJ = pk["NJ"]
    KC = NJ * 128
    quesT, keysT, valsP = pk["quesT"], pk["keysT"], pk["valsP"]
    wq_bf, wk_bf, wv_bf = pk["wq_bf"], pk["wk_bf"], pk["wv_bf"]
    indsig_bf, indsq_bf, indb_bf = pk["indsig_bf"], pk["indsq_bf"], pk["indb_bf"]
    ones_bf, guard, gobo, out_d, GS = (
        pk["ones_bf"], pk["guard"], pk["gobo"], pk["out_d"], pk["GS"])
    eps_t = pk["eps_t"]
    ep_dram = pk["ep_dram"]

    with tc.tile_pool(name="work", bufs=1) as wk:

        # ================= phase 1: projections + LN stats =================
        with tc.tile_pool(name="ph1sb", bufs=1) as sb1:
            quesT_bf = wk.tile([128, LQ], BF16)
            nc.vector.tensor_copy(quesT_bf[:], quesT[:])
            keysT_bf = sb1.tile([128, KC], BF16)
            nc.vector.tensor_copy(keysT_bf[:], keysT[:])
            valsP_bf = wk.tile([128, NJ, 128], BF16)
            nc.vector.tensor_copy(valsP_bf[:], valsP[:])

            def proj_stats(src_bf, W_bf, L, sig_scale, tg):
                """Row-packed projection; returns (proj_bf, var rows [4, L])."""
                with tc.tile_pool(name=f"pps{tg}", bufs=1, space="PSUM") as ps1:
                    proj_ps = ps1.tile([128, L], F32, tag=f"proj{tg}")
                    nc.vector.memset(proj_ps[:], 0.0)
                    for b in range(4):
                        for t0 in range(0, L, 512):
                            w = min(512, L - t0)
                            nc.tensor.matmul(
                                proj_ps[32 * b:32 * b + CAUG, t0:t0 + w],
                                W_bf[32 * b:32 * b + D, :],
                                src_bf[32 * b:32 * b + D, t0:t0 + w],
                                start=True, stop=True,
                                tile_position=(32 * b, 32 * b),
                            )
                    proj_bf = wk.tile([128, L], BF16, tag=f"projbf{tg}")
                    nc.vector.tensor_copy(proj_bf[:], proj_ps[:])
                sq_bf = sb1.tile([128, L], BF16, tag=f"sq{tg}")
                nc.vector.tensor_tensor(sq_bf[:], proj_bf[:], proj_bf[:], AO.mult)
                with tc.tile_pool(name=f"sps{tg}", bufs=1, space="PSUM") as ps2:
                    stat_ps = ps2.tile([64, L], F32, tag=f"stat{tg}")
                    for t0 in range(0, L, 512):
                        w = min(512, L - t0)
                        nc.tensor.matmul(stat_ps[0:4, t0:t0 + w],
                                         indsig_bf[:], proj_bf[:, t0:t0 + w],
                                         start=True, stop=True)
                        nc.tensor.matmul(stat_ps[32:36, t0:t0 + w],
                                         indsq_bf[:], sq_bf[:, t0:t0 + w],
                                         start=True, stop=True,
                                         tile_position=(0, 32))
                    rows_sig = sb1.tile([4, L], F32, tag=f"rsig{tg}")
                    nc.scalar.copy(rows_sig[:], stat_ps[0:4, :])
                    rows_sq = sb1.tile([4, L], F32, tag=f"rsq{tg}")
                    nc.scalar.copy(rows_sq[:], stat_ps[32:36, :])
                mu = sb1.tile([4, L], F32, tag=f"mu{tg}")
                nc.scalar.mul(mu[:], rows_sig[:], sig_scale / C)
                musq = sb1.tile([4, L], F32, tag=f"musq{tg}")
                nc.vector.tensor_tensor(musq[:], mu[:], mu[:], AO.mult)
                var = sb1.tile([4, L], F32, tag=f"var{tg}")
                nc.vector.scalar_tensor_tensor(
                    var[:], rows_sq[:], 1.0 / C, musq[:], AO.mult, AO.subtract)
                return proj_bf, var

            qhat_bf, var_q = proj_stats(quesT_bf, wq_bf, LQ, 1.0, "q")
            khat_bf, var_k = proj_stats(keysT_bf, wk_bf, KC, -1.0, "k")
            pk["khat_bf"] = khat_bf

            # batched Ln then batched Exp (2 ACT table loads total)
            lnq = sb1.tile([4, LQ], F32)
            nc.scalar.activation(lnq[:], var_q[:], AF.Ln, bias=eps_t[:])
            lnk = sb1.tile([4, KC], F32)
            nc.scalar.activation(lnk[:], var_k[:], AF.Ln, bias=eps_t[:])
            rq_bf = sb1.tile([4, LQ], BF16)
            nc.scalar.activation(rq_bf[:], lnq[:], AF.Exp, scale=-0.5)
            rk_f = sb1.tile([4, KC], F32)
            nc.scalar.activation(rk_f[:], lnk[:], AF.Exp, scale=-0.5)

            # broadcast r_q to strips via DRAM bounce; fold into q side
            rq_bc = sb1.tile([128, LQ], BF16)
            with tc.tile_pool(name="dramq", bufs=1, space="DRAM") as dramq:
                rq_dram = dramq.tile([4, LQ], BF16, tag="rqd")
                nc.sync.dma_start(rq_dram[:], rq_bf[0:4, :])
                for b in range(4):
                    nc.sync.dma_start(
                        rq_bc[32 * b:32 * b + CAUG, :],
                        rq_dram[b:b + 1, :].broadcast_to([CAUG, LQ]))
            qsc_bf = wk.tile([128, LQ], BF16)
            nc.vector.tensor_tensor(qsc_bf[:], qhat_bf[:], rq_bc[:], AO.mult)

            # r_k rows -> per-chunk column layout via DRAM bounce
            rk_cols = wk.tile([128, 4 * NJ], F32, tag="rk_cols")
            with tc.tile_pool(name="dramsc", bufs=1, space="DRAM") as dramp:
                rk_dram = dramp.tile([4, KC], F32, tag="rkd")
                nc.sync.dma_start(rk_dram[:], rk_f[0:4, :])
                for b in range(4):
                    nc.sync.dma_start(
                        rk_cols[:, NJ * b:NJ * (b + 1)],
                        rk_dram[b].rearrange("(j p) -> p j", p=128))
            sc_act = wk.tile([128, 4 * NJ], F32)   # rk * GS * guard
            nc.vector.tensor_tensor(sc_act[:], rk_cols[:], guard[:], AO.mult)
            nc.vector.tensor_scalar(sc_act[:], sc_act[:], GS, None, AO.mult)
            bias_act = wk.tile([128, 4 * NJ], F32)  # 0 real / -60 pad
            nc.vector.tensor_scalar(bias_act[:], guard[:], -ACT_PAD_BIAS,
                                    ACT_PAD_BIAS, AO.mult, AO.add)
            sc_dve = wk.tile([128, 4 * NJ], F32)   # rk * GS * A16 * guard
            nc.vector.tensor_scalar(sc_dve[:], sc_act[:], A16, None, AO.mult)
            b_dve = wk.tile([128, 4 * NJ], F32)    # B16 real / B16_PAD pad
            nc.vector.tensor_scalar(b_dve[:], guard[:], B16 - B16_PAD, B16_PAD,
                                    AO.mult, AO.add)

        # ================= phase 2: attention =================
        o_bfs = []
        obfp = wk  # o_bf tiles are consumed in phase 3; keep them body-scoped
        with tc.tile_pool(name="scps", bufs=4, space="PSUM") as scps, \
             tc.tile_pool(name="oacc", bufs=2, space="PSUM") as oaccp, \
             tc.tile_pool(name="sacc", bufs=2, space="PSUM") as saccp, \
             tc.tile_pool(name="psb", bufs=10) as psb, \
             tc.tile_pool(name="sumsp", bufs=2) as sumsp:
            for qt in range(NQT):
                t0 = qt * NT
                o_acc = oaccp.tile([128, NT], F32, tag="o")
                s_acc = saccp.tile([128, NT], F32, tag="s")
                for j in range(NJ):
                    # two scores banks, each holds 2 batches at free offsets
                    sc0 = scps.tile([128, 2 * NT], F32, tag="sc")
                    sc1 = scps.tile([128, 2 * NT], F32, tag="sc")
                    sc_slices = [sc0[:, 0:NT], sc0[:, NT:2 * NT],
                                 sc1[:, 0:NT], sc1[:, NT:2 * NT]]
                    p_tiles = []
                    for b in range(4):
                        s_ps = sc_slices[b]
                        nc.tensor.matmul(
                            s_ps,
                            pk["khat_bf"][32 * b:32 * b + CAUG,
                                          128 * j:128 * (j + 1)],
                            qsc_bf[32 * b:32 * b + CAUG, t0:t0 + NT],
                            start=True, stop=True,
                            tile_position=(32 * b, 0),
                        )
                        col = NJ * b + j
                        if _assign_dve(qt, j, b):
                            p_i16 = psb.tile([128, NT], I16, tag="p")
                            nc.vector.tensor_scalar(
                                p_i16[:], s_ps,
                                sc_dve[:, col:col + 1], b_dve[:, col:col + 1],
                                AO.mult, AO.add)
                            p_bf = p_i16[:].bitcast(BF16)
                        else:
                            p_t = psb.tile([128, NT], BF16, tag="p")
                            nc.scalar.activation(
                                p_t[:], s_ps, AF.Exp,
                                bias=bias_act[:, col:col + 1],
                                scale=sc_act[:, col:col + 1])
                            p_bf = p_t[:]
                        p_tiles.append(p_bf)
                    st, sp = (j == 0), (j == NJ - 1)
                    for b in range(4):
                        nc.tensor.matmul(
                            o_acc[32 * b:32 * b + 32, :],
                            valsP_bf[:, j, 32 * b:32 * b + 32],
                            p_tiles[b],
                            start=st, stop=sp, tile_position=(0, 32 * b))
                    for b in range(4):
                        nc.tensor.matmul(
                            s_acc[32 * b:32 * b + 32, :],
                            ones_bf[:],
                            p_tiles[b],
                            start=st, stop=sp, tile_position=(0, 32 * b))

                # stash o (bf16) + sums rows (via DRAM) for the finalize phase
                o_bf = obfp.tile([128, NT], BF16, tag=f"obf{qt}")
                nc.vector.tensor_copy(o_bf[:], o_acc[:])
                o_bfs.append(o_bf)
                sums = sumsp.tile([128, NT], F32, tag="sums")
                nc.scalar.copy(sums[:], s_acc[:])
                for b in range(4):
                    nc.sync.dma_start(ep_dram[0, b:b + 1, t0:t0 + NT],
                                      sums[32 * b:32 * b + 1, :])

        # ================= phase 3: output LN finalize =================
        with tc.tile_pool(name="ep", bufs=2) as ep, \
             tc.tile_pool(name="zp", bufs=NQT + 1) as zp, \
             tc.tile_pool(name="eprow", bufs=1) as eprow, \
             tc.tile_pool(name="epps", bufs=2, space="PSUM") as epps, \
             tc.tile_pool(name="stps", bufs=2, space="PSUM") as stps:
            zs = []
            srow_z = eprow.tile([4, LQ], F32)
            srow_z2 = eprow.tile([4, LQ], F32)
            for qt in range(NQT):
                t0 = qt * NT
                z1_ps = epps.tile([128, NT], F32, tag="z1")
                for b in range(4):
                    nc.tensor.matmul(
                        z1_ps[32 * b:32 * b + 32, :],
                        wv_bf[32 * b:32 * b + 32, :],
                        o_bfs[qt][32 * b:32 * b + 32, :],
                        start=True, stop=True,
                        tile_position=(32 * b, 32 * b))
                z1 = ep.tile([128, NT], F32, tag="z1sb")
                nc.scalar.copy(z1[:], z1_ps[:])
                s_bc = ep.tile([128, NT], F32, tag="sbc")
                for b in range(4):
                    nc.sync.dma_start(
                        s_bc[32 * b:32 * b + 32, :],
                        ep_dram[0, b:b + 1, t0:t0 + NT].broadcast_to([32, NT]))
                t1 = ep.tile([128, NT], F32, tag="t1")
                nc.vector.tensor_tensor(t1[:], quesT[:, t0:t0 + NT], s_bc[:],
                                        AO.mult)
                z = zp.tile([128, NT], F32, tag="z")
                nc.vector.tensor_tensor(z[:], t1[:], z1[:], AO.add)
                zs.append(z)
                z_bf = ep.tile([128, NT], BF16, tag="zbf")
                nc.vector.tensor_copy(z_bf[:], z[:])
                zsq_bf = ep.tile([128, NT], BF16, tag="zsq")
                nc.vector.tensor_tensor(zsq_bf[:], z_bf[:], z_bf[:], AO.mult)
                st_ps = stps.tile([64, NT], F32, tag="st")
                nc.tensor.matmul(st_ps[0:4, :], indb_bf[:], z_bf[:],
                                 start=True, stop=True)
                nc.tensor.matmul(st_ps[32:36, :], indb_bf[:], zsq_bf[:],
                                 start=True, stop=True, tile_position=(0, 32))
                nc.scalar.copy(srow_z[:, t0:t0 + NT], st_ps[0:4, :])
                nc.scalar.copy(srow_z2[:, t0:t0 + NT], st_ps[32:36, :])

            mu = eprow.tile([4, LQ], F32)
            nc.scalar.mul(mu[:], srow_z[:], 1.0 / D)
            musq = eprow.tile([4, LQ], F32)
            nc.vector.tensor_tensor(musq[:], mu[:], mu[:], AO.mult)
            var = eprow.tile([4, LQ], F32)
            nc.vector.scalar_tensor_tensor(
                var[:], srow_z2[:], 1.0 / D, musq[:], AO.mult, AO.subtract)
            lnv = eprow.tile([4, LQ], F32)
            nc.scalar.activation(lnv[:], var[:], AF.Ln, bias=eps_t[:])
            rstd = eprow.tile([4, LQ], F32)
            nc.scalar.activation(rstd[:], lnv[:], AF.Exp, scale=-0.5)
            nc.sync.dma_start(ep_dram[1, :, :], mu[:])
            nc.sync.dma_start(ep_dram[2, :, :], rstd[:])
            for qt in range(NQT):
                t0 = qt * NT
                mu_bc = ep.tile([128, NT], F32, tag="mubc")
                rstd_bc = ep.tile([128, NT], F32, tag="rstdbc")
                for b in range(4):
                    nc.sync.dma_start(
                        mu_bc[32 * b:32 * b + 32, :],
                        ep_dram[1, b:b + 1, t0:t0 + NT].broadcast_to([32, NT]))
                    nc.sync.dma_start(
                        rstd_bc[32 * b:32 * b + 32, :],
                        ep_dram[2, b:b + 1, t0:t0 + NT].broadcast_to([32, NT]))
                d1 = ep.tile([128, NT], F32, tag="d1")
                nc.vector.tensor_tensor(d1[:], zs[qt][:], mu_bc[:], AO.subtract)
                d2 = ep.tile([128, NT], F32, tag="d2")
                nc.vector.tensor_tensor(d2[:], d1[:], rstd_bc[:], AO.mult)
                zo = ep.tile([128, NT], F32, tag="zo")
                nc.vector.tensor_scalar(zo[:], d2[:], gobo[:, 0:1], gobo[:, 1:2],
                                        AO.mult, AO.add)
                nc.sync.dma_start(out_d[:, t0:t0 + NT], zo[:])


# ---------------------------------------------------------------------------
# host side
# ---------------------------------------------------------------------------

def prepare_inputs(vals, keys, ques, key_mask, W_v, W_k, W_q,
                   g_k, b_k, g_q, b_q, g_o, b_o):
    """Shard + lay out the full inputs for the 8 cores. Returns (in_maps, KC)."""
    vals = np.ascontiguousarray(vals, np.float32)
    keys = np.ascontiguousarray(keys, np.float32)
    ques = np.ascontiguousarray(ques, np.float32)
    key_mask = np.asarray(key_mask)
    W_v = np.asarray(W_v, np.float32)
    W_k = np.asarray(W_k, np.float32)
    W_q = np.asarray(W_q, np.float32)
    g_k = np.asarray(g_k, np.float32)
    b_k = np.asarray(b_k, np.float32)
    g_q = np.asarray(g_q, np.float32)
    b_q = np.asarray(b_q, np.float32)
    g_o = np.asarray(g_o, np.float32)
    b_o = np.asarray(b_o, np.float32)

    # supported parameterization (holds for the harness inputs)
    if not (np.allclose(b_k, 0) and np.allclose(b_q, 0)):
        raise NotImplementedError("nonzero k/q LN bias not supported")
    if not (np.allclose(g_k, g_k.flat[0]) and np.allclose(g_q, g_q.flat[0])):
        raise NotImplementedError("non-uniform k/q LN gain not supported")
    guni = float(g_k.flat[0] * g_q.flat[0])

    counts = (~key_mask).sum(axis=1)
    KC = int(np.ceil(max(int(counts.max()), 1) / 128) * 128)
    NJ = KC // 128

    s20 = math.sqrt(C)
    wq_aug = np.zeros((32, CAUG), np.float32)
    wq_aug[:, :C] = W_q.T
    wq_aug[:, C] = W_q.sum(axis=0) / s20
    wk_aug = np.zeros((32, CAUG), np.float32)
    wk_aug[:, :C] = W_k.T
    wk_aug[:, C] = -W_k.sum(axis=0) / s20

    wq_st = np.zeros((128, CAUG), np.float32)
    wk_st = np.zeros((128, CAUG), np.float32)
    wv_st = np.zeros((128, D), np.float32)
    indsig = np.zeros((128, BPC), np.float32)
    indsq = np.zeros((128, BPC), np.float32)
    indb = np.zeros((128, BPC), np.float32)
    go_bo = np.zeros((128, 2), np.float32)
    for b in range(BPC):
        wq_st[32 * b:32 * b + 32] = wq_aug
        wk_st[32 * b:32 * b + 32] = wk_aug
        wv_st[32 * b:32 * b + 32] = W_v.T
        indsig[32 * b + C, b] = s20
        indsq[32 * b:32 * b + C, b] = 1.0
        indb[32 * b:32 * b + 32, b] = 1.0
        go_bo[32 * b:32 * b + 32, 0] = g_o
        go_bo[32 * b:32 * b + 32, 1] = b_o
    # fold uniform gain into the score scale via wq (GS stays 1/sqrt(C))
    wq_st *= guni

    in_maps = []
    for c in range(NCORES):
        quesT = np.zeros((128, LQ), np.float32)
        keysT = np.zeros((128, KC), np.float32)
        valsP = np.zeros((128, NJ * 128), np.float32)
        guard = np.zeros((128, BPC * NJ), np.float32)
        for b in range(BPC):
            g = c * BPC + b
            idx = np.flatnonzero(~key_mask[g])
            ci = len(idx)
            quesT[32 * b:32 * b + 32] = ques[g].T
            keysT[32 * b:32 * b + 32, :ci] = keys[g][idx].T
            vc = np.zeros((KC, D), np.float32)
            vc[:ci] = vals[g][idx]
            for j in range(NJ):
                valsP[:, 128 * j + 32 * b:128 * j + 32 * b + 32] = \
                    vc[128 * j:128 * (j + 1)]
            gcol = np.zeros((KC,), np.float32)
            gcol[:ci] = 1.0
            guard[:, NJ * b:NJ * (b + 1)] = gcol.reshape(NJ, 128).T
        in_maps.append({
            "quesT": quesT, "keysT": keysT, "valsP": valsP,
            "wq_st": wq_st, "wk_st": wk_st, "wv_st": wv_st,
            "ind_sig": indsig, "ind_sq": indsq, "ind_b": indb,
            "guard": guard, "go_bo": go_bo,
            "ones_in": np.concatenate([np.ones((128, 1), np.float32),
                                       np.zeros((128, 31), np.float32)], axis=1),
        })
    return in_maps, KC


def unshard_output(results):
    out = np.empty((B, LQ, D), np.float32)
    for c in range(NCORES):
        o = results[c]["out"]
        for b in range(BPC):
            out[c * BPC + b] = o[32 * b:32 * b + 32, :].T
    return out


def kernel(**inputs) -> np.ndarray:
    in_maps, KC = prepare_inputs(**inputs)
    key = ("nc", KC)
    if key not in _cache:
        _cache[key] = build_module(KC)
    nc = _cache[key]
    res = bass_utils.run_bass_kernel_spmd(nc, in_maps,
                                          core_ids=list(range(NCORES)))
    return unshard_output(res.results)
